# revision 30
# baseline (speedup 1.0000x reference)
"""Trainium2 Bass kernel for nn_MixtureOfSoftMaxACF (mixture-of-softmax attention).

Reference computation per batch element b (B=8, one per NeuronCore):
    pi      = softmax(weight @ mean(qt[b], axis=-1))                      # [m=2]
    A_j     = q_j^T k_j / sqrt(d_k)       (q_j, k_j = d=256-row slices)   # [N, N2]
    attn    = sum_j pi_j * softmax(A_j, axis=-1)                          # [N, N2]
    out     = attn @ vt[b]^T                                              # [N, d_v]

Sharding: data-parallel over B across the 8 cores. Inside each core:
  - QK^T runs in bf16 with native [d_k, N] layouts (lhsT=q, rhs=k).
  - exp on ScalarE with accum_out giving row sums for free (no max
    subtraction needed: |logits| <= ~6).
  - mixture weights pi computed on-device from the q-cast accumulation.
  - mixing on VectorE with per-partition scalars r_j = pi_j / S_j.
  - attn is transposed (DMA xbar or PE) to feed the attn @ v matmul.
"""

import sys

if "/opt/trn_rl_repo" not in sys.path:
    sys.path.insert(0, "/opt/trn_rl_repo")

import numpy as np
from contextlib import ExitStack

import concourse.bacc as bacc
import concourse.bass as bass
import concourse.tile as tile
from concourse import mybir
from concourse.bass_utils import run_bass_kernel_spmd
from concourse.masks import make_identity

B, DK, N = 8, 512, 2048
M = 2                       # mixture components
D = DK // M                 # 256 per-mixture head dim
TEMP = float(DK) ** 0.5     # sqrt(d_k), matching the reference
NCH = N // 128              # 16 n-chunks (and n2-chunks)
DCH = DK // 128             # 4 d_k partition chunks

f32 = mybir.dt.float32
bf16 = mybir.dt.bfloat16
f8 = mybir.dt.float8e4
F8_SCALE = 64.0  # lift attn probs out of fp8e4m3 subnormals; attn <= 1 so 64x never overflows
Exp = mybir.ActivationFunctionType.Exp
Copy = mybir.ActivationFunctionType.Copy
MULT = mybir.AluOpType.mult
ADD = mybir.AluOpType.add

TRANSPOSE_MODE = "pe"     # "xbar" (DMA xbar transpose) or "pe" (TensorE transpose)

_NC_CACHE = {}


def _body(ctx, tc, qt, kt, vtT, wT, out_d, attn_d, mode):
    nc = tc.nc

    const = ctx.enter_context(tc.tile_pool(name="const", bufs=1))
    resid = ctx.enter_context(tc.tile_pool(name="resid", bufs=1))
    stage = ctx.enter_context(tc.tile_pool(name="stage", bufs=2))
    stage_k = ctx.enter_context(tc.tile_pool(name="stage_k", bufs=3))
    stage_q = ctx.enter_context(tc.tile_pool(name="stage_q", bufs=4))
    epool = ctx.enter_context(tc.tile_pool(name="epool", bufs=6))
    apool = ctx.enter_context(tc.tile_pool(name="apool", bufs=3))
    spool = ctx.enter_context(tc.tile_pool(name="spool", bufs=6))
    opool = ctx.enter_context(tc.tile_pool(name="opool", bufs=3))
    # PSUM: psA 2 halves x [128,1024] (2 banks each) = 4 banks,
    # psT [128,512] = 1-2 banks, psU [128,512] x2 = 2 banks.
    psA_p = ctx.enter_context(tc.tile_pool(name="psA", bufs=2, space="PSUM"))
    psT_p = ctx.enter_context(
        tc.tile_pool(name="psT", bufs=(2 if mode == "pe" else 1), space="PSUM")
    )
    psU_p = ctx.enter_context(tc.tile_pool(name="psU", bufs=2, space="PSUM"))

    # ---- load + cast inputs ----
    # k loads first (every chunk needs all of k as the moving operand),
    # in column halves so chunk 0's first matmuls start ~5 us in. q loads
    # as 256-column blocks just-in-time for successive chunks, cast on
    # DVE with accum partials feeding the mixture-weight logits.
    q_bf = resid.tile([128, DCH, N], bf16)
    k_bf = resid.tile([128, DCH, N], bf16)
    qp = spool.tile([128, DCH, 8], f32, tag="qp")
    qr = qt[:, :].rearrange("(c p) n -> p c n", p=128)

    def _load_k_half(c, h):
        st = stage_k.tile([128, 1024], f32, tag="khalf", name=f"stk_{c}_{h}")
        nc.sync.dma_start(st, kt[bass.ts(c, 128), bass.ts(h, 1024)])
        nc.vector.tensor_copy(k_bf[:, c, bass.ts(h, 1024)], st)

    def _load_qcol(b):
        st = stage_q.tile([128, DCH, 256], f32, tag="qcol", name=f"stq_{b}")
        nc.sync.dma_start(st, qr[:, :, bass.ts(b, 256)])
        for c in range(DCH):
            nc.vector.tensor_scalar(
                q_bf[:, c, bass.ts(b, 256)], st[:, c, :], 1.0, 0.0, MULT, ADD,
                accum_out=qp[:, c, b : b + 1],
            )

    def _load_k_quarter(c, qtr):
        st = stage_k.tile([128, 512], f32, tag="kq", name=f"stkq_{c}_{qtr}")
        nc.sync.dma_start(st, kt[bass.ts(c, 128), bass.ts(qtr, 512)])
        nc.vector.tensor_copy(k_bf[:, c, bass.ts(qtr, 512)], st)

    vT_bf = resid.tile([128, NCH, DK], bf16)
    vr = vtT[:, :].rearrange("(g c p) d -> g p c d", g=4, p=128)

    def _load_v(g):
        st = stage.tile([128, 4, DK], f32, tag="stage", name=f"stv_{g}")
        nc.sync.dma_start(st, vr[g])
        nc.vector.tensor_copy(vT_bf[:, g * 4 : (g + 1) * 4, :], st)

    # smallest first pieces so the first QK matmuls start ~10 us in
    _load_k_quarter(0, 0)
    _load_k_quarter(1, 0)
    _load_qcol(0)
    _load_k_quarter(0, 1)
    _load_k_quarter(1, 1)
    for c in (2, 3):
        _load_k_half(c, 0)
    for c in (0, 1, 2, 3):
        _load_k_half(c, 1)
    for b in range(1, 8):
        _load_qcol(b)
    for g in range(4):
        _load_v(g)

    qs = spool.tile([128, DCH], f32, tag="qs")
    nc.vector.reduce_sum(qs, qp, axis=mybir.AxisListType.X)

    wT_sb = const.tile([128, DCH, M], f32)
    nc.sync.dma_start(wT_sb, wT[:, :].rearrange("(c p) m -> p c m", p=128))

    # ---- mixture weights pi ----
    # logits[m] = sum_d qsum[d] * wT[d, m]  (qsum = N * bar_q)
    lg_ps = psT_p.tile([128, 512], f32, tag="psT")
    for c in range(DCH):
        nc.tensor.matmul(
            lg_ps[:1, :M],
            lhsT=qs[:, c : c + 1],
            rhs=wT_sb[:, c, :],
            start=(c == 0),
            stop=(c == DCH - 1),
        )
    e_pi = spool.tile([1, M], f32, tag="epi")
    s_pi = spool.tile([1, 1], f32, tag="spi")
    # pi = softmax(logits / N); |logits/N| is tiny so no max subtraction
    nc.scalar.activation(e_pi, lg_ps[:1, :M], Exp, scale=1.0 / float(N), accum_out=s_pi)
    rs_pi = spool.tile([1, 1], f32, tag="rspi")
    nc.vector.reciprocal(rs_pi, s_pi)
    pi_row = spool.tile([1, M], f32, tag="pirow")
    nc.vector.tensor_scalar_mul(pi_row, e_pi, rs_pi)
    # broadcast pi to all 128 partitions via ones-matmul
    ones_r = const.tile([1, 128], f32)
    nc.vector.memset(ones_r, 1.0)
    pi_ps = psT_p.tile([128, 512], f32, tag="psT")
    nc.tensor.matmul(pi_ps[:, :M], lhsT=ones_r, rhs=pi_row, start=True, stop=True)
    pi_bc = const.tile([128, M], f32)
    nc.vector.tensor_copy(pi_bc, pi_ps[:, :M])

    if mode == "pe":
        ident = const.tile([128, 128], bf16)
        make_identity(nc, ident)

    # ---- main loop over 128-row n-chunks ----
    # Emission interleaves chunk ni's QK matmuls with chunk ni-1's
    # transpose+AV matmuls so the PE stream always has filler while the
    # ScalarE exps drain the QK PSUM tiles, and each transpose's
    # LDWEIGHTS hides behind an AV matmul's 512-col stream.
    prev = None

    def _emit_qk(ni, j, S2):
        e_t = epool.tile([128, N], bf16, tag="E", name=f"e_{ni}_{j}")
        for h in range(2):
            pa = psA_p.tile([128, 1024], f32, tag="psA", name=f"pa_{ni}_{j}_{h}")
            for d in range(2):
                dch = j * 2 + d
                lhsT = q_bf[:, dch, bass.ts(ni, 128)]
                for s in range(2):
                    nc.tensor.matmul(
                        pa[:, bass.ts(s, 512)],
                        lhsT=lhsT,
                        rhs=k_bf[:, dch, h * 1024 + s * 512 : h * 1024 + (s + 1) * 512],
                        start=(d == 0),
                        stop=(d == 1),
                    )
            nc.scalar.activation(
                e_t[:, bass.ts(h, 1024)], pa, Exp, scale=1.0 / TEMP,
                accum_out=S2[:, j : j + 1] if h == 1 else S2[:, 2 + j : 3 + j],
            )
        return e_t

    def _emit_mix(ni, E, S2):
        # S = partial(h0) + partial(h1); r_j = pi_j / S_j
        S = spool.tile([128, M], f32, tag="S", name=f"S_{ni}")
        nc.vector.tensor_add(S, S2[:, 0:M], S2[:, M : 2 * M])
        rS = spool.tile([128, M], f32, tag="rS", name=f"rS_{ni}")
        nc.vector.reciprocal(rS, S)
        r2 = spool.tile([128, M], f32, tag="r2", name=f"r2_{ni}")
        nc.vector.tensor_mul(r2, rS, pi_bc)
        t0_bf = apool.tile([128, N], bf16, tag="t0bf", name=f"t0_{ni}")
        nc.vector.tensor_scalar_mul(t0_bf, E[0], r2[:, 0:1])
        t_bf = apool.tile([128, N], bf16, tag="tbf", name=f"t1_{ni}")
        nc.vector.tensor_scalar_mul(t_bf, E[1], r2[:, 1:2])
        a_bf = apool.tile([128, N], bf16, tag="abf", name=f"abf_{ni}")
        nc.vector.tensor_add(a_bf, t0_bf, t_bf)
        a_f32 = apool.tile([128, N], f32, tag="af32", name=f"af32_{ni}")
        nc.vector.tensor_copy(a_f32, a_bf)
        nc.sync.dma_start(attn_d[bass.ts(ni, 128), :], a_f32)
        return a_bf

    class Chunk:
        pass

    def _emit_tav(ck, part):
        # part 0: transpose groups 0,1 + AV 0..3 ; part 1: the rest
        if part == 0:
            for g in (0, 1):
                pt = psT_p.tile([128, 512], bf16, tag="psT", name=f"pt_{ck.ni}_{g}")
                for kk in range(4):
                    nc.tensor.transpose(
                        pt[:, bass.ts(kk, 128)],
                        ck.a_bf[:, bass.ts(g * 4 + kk, 128)], ident,
                    )
                nc.vector.tensor_copy(ck.aT[:, bass.ts(g, 512)], pt)
            for kb in range(4):
                _av(ck, kb)
        else:
            for g in (2, 3):
                pt = psT_p.tile([128, 512], bf16, tag="psT", name=f"pt_{ck.ni}_{g}")
                for kk in range(4):
                    nc.tensor.transpose(
                        pt[:, bass.ts(kk, 128)],
                        ck.a_bf[:, bass.ts(g * 4 + kk, 128)], ident,
                    )
                nc.vector.tensor_copy(ck.aT[:, bass.ts(g, 512)], pt)
                for kb in range((g - 1) * 4, g * 4):
                    _av(ck, kb)
            for kb in range(12, 16):
                _av(ck, kb)
            o_t = opool.tile([128, DK], f32, tag="ot", name=f"ot_{ck.ni}")
            nc.scalar.copy(o_t, ck.pu)
            nc.sync.dma_start(out_d[bass.ts(ck.ni, 128), :], o_t)

    def _av(ck, kb):
        nc.tensor.matmul(
            ck.pu,
            lhsT=ck.aT[:, bass.ts(kb, 128)],
            rhs=vT_bf[:, kb, :],
            start=(kb == 0),
            stop=(kb == NCH - 1),
        )

    for ni in range(NCH):
        S2 = spool.tile([128, 2 * M], f32, tag="S2", name=f"S2_{ni}")
        E = [_emit_qk(ni, 0, S2)]
        if prev is not None:
            _emit_tav(prev, 0)
        E.append(_emit_qk(ni, 1, S2))
        if prev is not None:
            _emit_tav(prev, 1)
        a_bf = _emit_mix(ni, E, S2)
        ck = Chunk()
        ck.ni = ni
        ck.a_bf = a_bf
        ck.aT = apool.tile([128, N], bf16, tag="aT", name=f"aT_{ni}")
        ck.pu = psU_p.tile([128, DK], f32, tag="psU", name=f"pu_{ni}")
        prev = ck

    _emit_tav(prev, 0)
    _emit_tav(prev, 1)


def _build(mode):
    nc = bacc.Bacc()
    qt = nc.dram_tensor("qt", [DK, N], f32, kind="ExternalInput")
    kt = nc.dram_tensor("kt", [DK, N], f32, kind="ExternalInput")
    vtT = nc.dram_tensor("vtT", [N, DK], f32, kind="ExternalInput")
    wT = nc.dram_tensor("wT", [DK, M], f32, kind="ExternalInput")
    out_d = nc.dram_tensor("out", [N, DK], f32, kind="ExternalOutput")
    attn_d = nc.dram_tensor("attn", [N, N], f32, kind="ExternalOutput")
    with tile.TileContext(nc) as tc, ExitStack() as ctx:
        _body(ctx, tc, qt, kt, vtT, wT, out_d, attn_d, mode)
    nc.finalize()
    return nc


def _get_nc(mode=None):
    mode = mode or TRANSPOSE_MODE
    if mode not in _NC_CACHE:
        _NC_CACHE[mode] = _build(mode)
    return _NC_CACHE[mode]


def _in_maps(qt, kt, vt, weight):
    qt = np.asarray(qt, dtype=np.float32)
    kt = np.asarray(kt, dtype=np.float32)
    vt = np.asarray(vt, dtype=np.float32)
    wT = np.ascontiguousarray(np.asarray(weight, dtype=np.float32).T)  # [DK, M]
    maps = []
    for b in range(B):
        maps.append(
            {
                "qt": np.ascontiguousarray(qt[b]),
                "kt": np.ascontiguousarray(kt[b]),
                "vtT": np.ascontiguousarray(vt[b].T),  # [N2, d_v]
                "wT": wT,
            }
        )
    return maps


def kernel_with_result(qt, kt, vt, weight, **run_kwargs):
    """Like kernel() but also returns the BassKernelResults (for profiling)."""
    nc = _get_nc()
    res = run_bass_kernel_spmd(nc, _in_maps(qt, kt, vt, weight),
                               core_ids=list(range(B)), **run_kwargs)
    output = np.stack([np.asarray(res.results[b]["out"]) for b in range(B)])
    attn = np.stack([np.asarray(res.results[b]["attn"]) for b in range(B)])
    return (output.astype(np.float32, copy=False),
            attn.astype(np.float32, copy=False), res)


def kernel(qt, kt, vt, weight):
    """Full-input entry point: shards over B across 8 cores, returns
    (output [B, N, d_v], attn [B, N, N2]) matching the reference."""
    out, attn, _ = kernel_with_result(qt, kt, vt, weight)
    return out, attn


# revision 31
# speedup vs baseline: 1.0279x; 1.0279x over previous
"""Trainium2 Bass kernel for nn_MixtureOfSoftMaxACF (mixture-of-softmax attention).

Reference computation per batch element b (B=8, one per NeuronCore):
    pi      = softmax(weight @ mean(qt[b], axis=-1))                      # [m=2]
    A_j     = q_j^T k_j / sqrt(d_k)       (q_j, k_j = d=256-row slices)   # [N, N2]
    attn    = sum_j pi_j * softmax(A_j, axis=-1)                          # [N, N2]
    out     = attn @ vt[b]^T                                              # [N, d_v]

Sharding: data-parallel over B across the 8 cores. Inside each core:
  - QK^T runs in bf16 with native [d_k, N] layouts (lhsT=q, rhs=k).
  - exp on ScalarE with accum_out giving row sums for free (no max
    subtraction needed: |logits| <= ~6).
  - mixture weights pi computed on-device from the q-cast accumulation.
  - mixing on VectorE with per-partition scalars r_j = pi_j / S_j.
  - attn is transposed (DMA xbar or PE) to feed the attn @ v matmul.
"""

import sys

if "/opt/trn_rl_repo" not in sys.path:
    sys.path.insert(0, "/opt/trn_rl_repo")

import numpy as np
from contextlib import ExitStack

import concourse.bacc as bacc
import concourse.bass as bass
import concourse.tile as tile
from concourse import mybir
from concourse.bass_utils import run_bass_kernel_spmd
from concourse.masks import make_identity

B, DK, N = 8, 512, 2048
M = 2                       # mixture components
D = DK // M                 # 256 per-mixture head dim
TEMP = float(DK) ** 0.5     # sqrt(d_k), matching the reference
NCH = N // 128              # 16 n-chunks (and n2-chunks)
DCH = DK // 128             # 4 d_k partition chunks

f32 = mybir.dt.float32
bf16 = mybir.dt.bfloat16
f8 = mybir.dt.float8e4
F8_SCALE = 64.0  # lift attn probs out of fp8e4m3 subnormals; attn <= 1 so 64x never overflows
Exp = mybir.ActivationFunctionType.Exp
Copy = mybir.ActivationFunctionType.Copy
MULT = mybir.AluOpType.mult
ADD = mybir.AluOpType.add

TRANSPOSE_MODE = "pe"     # "xbar" (DMA xbar transpose) or "pe" (TensorE transpose)

_NC_CACHE = {}


def _body(ctx, tc, qt, kt, vtT, wT, out_d, attn_d, mode):
    nc = tc.nc

    const = ctx.enter_context(tc.tile_pool(name="const", bufs=1))
    resid = ctx.enter_context(tc.tile_pool(name="resid", bufs=1))
    stage = ctx.enter_context(tc.tile_pool(name="stage", bufs=2))
    stage_k = ctx.enter_context(tc.tile_pool(name="stage_k", bufs=4))
    stage_q = ctx.enter_context(tc.tile_pool(name="stage_q", bufs=4))
    epool = ctx.enter_context(tc.tile_pool(name="epool", bufs=6))
    apool = ctx.enter_context(tc.tile_pool(name="apool", bufs=3))
    spool = ctx.enter_context(tc.tile_pool(name="spool", bufs=6))
    opool = ctx.enter_context(tc.tile_pool(name="opool", bufs=3))
    # PSUM: psA 2 halves x [128,1024] (2 banks each) = 4 banks,
    # psT [128,512] = 1-2 banks, psU [128,512] x2 = 2 banks.
    psA_p = ctx.enter_context(tc.tile_pool(name="psA", bufs=2, space="PSUM"))
    psT_p = ctx.enter_context(
        tc.tile_pool(name="psT", bufs=(2 if mode == "pe" else 1), space="PSUM")
    )
    psU_p = ctx.enter_context(tc.tile_pool(name="psU", bufs=2, space="PSUM"))

    # ---- load + cast inputs ----
    # k loads first (every chunk needs all of k as the moving operand),
    # in column halves so chunk 0's first matmuls start ~5 us in. q loads
    # as 256-column blocks just-in-time for successive chunks, cast on
    # DVE with accum partials feeding the mixture-weight logits.
    q_bf = resid.tile([128, DCH, N], bf16)
    k_bf = resid.tile([128, DCH, N], bf16)
    qp = spool.tile([128, DCH, 8], f32, tag="qp")
    qr = qt[:, :].rearrange("(c p) n -> p c n", p=128)

    def _load_k_half(c, h):
        st = stage_k.tile([128, 1024], f32, tag="khalf", name=f"stk_{c}_{h}")
        nc.sync.dma_start(st, kt[bass.ts(c, 128), bass.ts(h, 1024)])
        nc.vector.tensor_copy(k_bf[:, c, bass.ts(h, 1024)], st)

    def _load_qcol(b):
        st = stage_q.tile([128, DCH, 256], f32, tag="qcol", name=f"stq_{b}")
        nc.sync.dma_start(st, qr[:, :, bass.ts(b, 256)])
        for c in range(DCH):
            nc.vector.tensor_scalar(
                q_bf[:, c, bass.ts(b, 256)], st[:, c, :], 1.0, 0.0, MULT, ADD,
                accum_out=qp[:, c, b : b + 1],
            )

    def _load_k_quarter(c, qtr):
        st = stage_k.tile([128, 512], f32, tag="kq", name=f"stkq_{c}_{qtr}")
        nc.sync.dma_start(st, kt[bass.ts(c, 128), bass.ts(qtr, 512)])
        nc.vector.tensor_copy(k_bf[:, c, bass.ts(qtr, 512)], st)

    vT_bf = resid.tile([128, NCH, DK], bf16)
    vr = vtT[:, :].rearrange("(g c p) d -> g p c d", g=4, p=128)

    def _load_v(g):
        st = stage.tile([128, 4, DK], f32, tag="stage", name=f"stv_{g}")
        nc.sync.dma_start(st, vr[g])
        nc.vector.tensor_copy(vT_bf[:, g * 4 : (g + 1) * 4, :], st)

    for c in (0, 1):
        _load_k_half(c, 0)
    _load_qcol(0)
    for c in (0, 1):
        _load_k_half(c, 1)
    for c in (2, 3):
        _load_k_half(c, 0)
    for c in (2, 3):
        _load_k_half(c, 1)
    for b in range(1, 8):
        _load_qcol(b)
    for g in range(4):
        _load_v(g)

    qs = spool.tile([128, DCH], f32, tag="qs")
    nc.vector.reduce_sum(qs, qp, axis=mybir.AxisListType.X)

    wT_sb = const.tile([128, DCH, M], f32)
    nc.sync.dma_start(wT_sb, wT[:, :].rearrange("(c p) m -> p c m", p=128))

    # ---- mixture weights pi ----
    # logits[m] = sum_d qsum[d] * wT[d, m]  (qsum = N * bar_q)
    lg_ps = psT_p.tile([128, 512], f32, tag="psT")
    for c in range(DCH):
        nc.tensor.matmul(
            lg_ps[:1, :M],
            lhsT=qs[:, c : c + 1],
            rhs=wT_sb[:, c, :],
            start=(c == 0),
            stop=(c == DCH - 1),
        )
    e_pi = spool.tile([1, M], f32, tag="epi")
    s_pi = spool.tile([1, 1], f32, tag="spi")
    # pi = softmax(logits / N); |logits/N| is tiny so no max subtraction
    nc.scalar.activation(e_pi, lg_ps[:1, :M], Exp, scale=1.0 / float(N), accum_out=s_pi)
    rs_pi = spool.tile([1, 1], f32, tag="rspi")
    nc.vector.reciprocal(rs_pi, s_pi)
    pi_row = spool.tile([1, M], f32, tag="pirow")
    nc.vector.tensor_scalar_mul(pi_row, e_pi, rs_pi)
    # broadcast pi to all 128 partitions via ones-matmul
    ones_r = const.tile([1, 128], f32)
    nc.vector.memset(ones_r, 1.0)
    pi_ps = psT_p.tile([128, 512], f32, tag="psT")
    nc.tensor.matmul(pi_ps[:, :M], lhsT=ones_r, rhs=pi_row, start=True, stop=True)
    pi_bc = const.tile([128, M], f32)
    nc.vector.tensor_copy(pi_bc, pi_ps[:, :M])

    if mode == "pe":
        ident = const.tile([128, 128], bf16)
        make_identity(nc, ident)

    # ---- main loop over 128-row n-chunks ----
    # Emission interleaves chunk ni's QK matmuls with chunk ni-1's
    # transpose+AV matmuls so the PE stream always has filler while the
    # ScalarE exps drain the QK PSUM tiles, and each transpose's
    # LDWEIGHTS hides behind an AV matmul's 512-col stream.
    prev = None

    def _emit_qk(ni, j, S2):
        e_t = epool.tile([128, N], bf16, tag="E", name=f"e_{ni}_{j}")
        for h in range(2):
            pa = psA_p.tile([128, 1024], f32, tag="psA", name=f"pa_{ni}_{j}_{h}")
            for d in range(2):
                dch = j * 2 + d
                lhsT = q_bf[:, dch, bass.ts(ni, 128)]
                for s in range(2):
                    nc.tensor.matmul(
                        pa[:, bass.ts(s, 512)],
                        lhsT=lhsT,
                        rhs=k_bf[:, dch, h * 1024 + s * 512 : h * 1024 + (s + 1) * 512],
                        start=(d == 0),
                        stop=(d == 1),
                    )
            nc.scalar.activation(
                e_t[:, bass.ts(h, 1024)], pa, Exp, scale=1.0 / TEMP,
                accum_out=S2[:, j : j + 1] if h == 1 else S2[:, 2 + j : 3 + j],
            )
        return e_t

    def _emit_mix(ni, E, S2):
        # S = partial(h0) + partial(h1); r_j = pi_j / S_j
        S = spool.tile([128, M], f32, tag="S", name=f"S_{ni}")
        nc.vector.tensor_add(S, S2[:, 0:M], S2[:, M : 2 * M])
        rS = spool.tile([128, M], f32, tag="rS", name=f"rS_{ni}")
        nc.vector.reciprocal(rS, S)
        r2 = spool.tile([128, M], f32, tag="r2", name=f"r2_{ni}")
        nc.vector.tensor_mul(r2, rS, pi_bc)
        t0_bf = apool.tile([128, N], bf16, tag="t0bf", name=f"t0_{ni}")
        nc.vector.tensor_scalar_mul(t0_bf, E[0], r2[:, 0:1])
        t_bf = apool.tile([128, N], bf16, tag="tbf", name=f"t1_{ni}")
        nc.vector.tensor_scalar_mul(t_bf, E[1], r2[:, 1:2])
        a_bf = apool.tile([128, N], bf16, tag="abf", name=f"abf_{ni}")
        nc.vector.tensor_add(a_bf, t0_bf, t_bf)
        a_f32 = apool.tile([128, N], f32, tag="af32", name=f"af32_{ni}")
        nc.vector.tensor_copy(a_f32, a_bf)
        nc.sync.dma_start(attn_d[bass.ts(ni, 128), :], a_f32)
        return a_bf

    class Chunk:
        pass

    def _emit_tav(ck, part):
        # part 0: transpose groups 0,1 + AV 0..3 ; part 1: the rest
        if part == 0:
            for g in (0, 1):
                pt = psT_p.tile([128, 512], bf16, tag="psT", name=f"pt_{ck.ni}_{g}")
                for kk in range(4):
                    nc.tensor.transpose(
                        pt[:, bass.ts(kk, 128)],
                        ck.a_bf[:, bass.ts(g * 4 + kk, 128)], ident,
                    )
                nc.vector.tensor_copy(ck.aT[:, bass.ts(g, 512)], pt)
            for kb in range(4):
                _av(ck, kb)
        else:
            for g in (2, 3):
                pt = psT_p.tile([128, 512], bf16, tag="psT", name=f"pt_{ck.ni}_{g}")
                for kk in range(4):
                    nc.tensor.transpose(
                        pt[:, bass.ts(kk, 128)],
                        ck.a_bf[:, bass.ts(g * 4 + kk, 128)], ident,
                    )
                nc.vector.tensor_copy(ck.aT[:, bass.ts(g, 512)], pt)
                for kb in range((g - 1) * 4, g * 4):
                    _av(ck, kb)
            for kb in range(12, 16):
                _av(ck, kb)
            o_t = opool.tile([128, DK], f32, tag="ot", name=f"ot_{ck.ni}")
            nc.scalar.copy(o_t, ck.pu)
            nc.sync.dma_start(out_d[bass.ts(ck.ni, 128), :], o_t)

    def _av(ck, kb):
        nc.tensor.matmul(
            ck.pu,
            lhsT=ck.aT[:, bass.ts(kb, 128)],
            rhs=vT_bf[:, kb, :],
            start=(kb == 0),
            stop=(kb == NCH - 1),
        )

    for ni in range(NCH):
        S2 = spool.tile([128, 2 * M], f32, tag="S2", name=f"S2_{ni}")
        E = [_emit_qk(ni, 0, S2)]
        if prev is not None:
            _emit_tav(prev, 0)
        E.append(_emit_qk(ni, 1, S2))
        if prev is not None:
            _emit_tav(prev, 1)
        a_bf = _emit_mix(ni, E, S2)
        ck = Chunk()
        ck.ni = ni
        ck.a_bf = a_bf
        ck.aT = apool.tile([128, N], bf16, tag="aT", name=f"aT_{ni}")
        ck.pu = psU_p.tile([128, DK], f32, tag="psU", name=f"pu_{ni}")
        prev = ck

    _emit_tav(prev, 0)
    _emit_tav(prev, 1)


def _build(mode):
    nc = bacc.Bacc()
    qt = nc.dram_tensor("qt", [DK, N], f32, kind="ExternalInput")
    kt = nc.dram_tensor("kt", [DK, N], f32, kind="ExternalInput")
    vtT = nc.dram_tensor("vtT", [N, DK], f32, kind="ExternalInput")
    wT = nc.dram_tensor("wT", [DK, M], f32, kind="ExternalInput")
    out_d = nc.dram_tensor("out", [N, DK], f32, kind="ExternalOutput")
    attn_d = nc.dram_tensor("attn", [N, N], f32, kind="ExternalOutput")
    with tile.TileContext(nc) as tc, ExitStack() as ctx:
        _body(ctx, tc, qt, kt, vtT, wT, out_d, attn_d, mode)
    nc.finalize()
    return nc


def _get_nc(mode=None):
    mode = mode or TRANSPOSE_MODE
    if mode not in _NC_CACHE:
        _NC_CACHE[mode] = _build(mode)
    return _NC_CACHE[mode]


def _in_maps(qt, kt, vt, weight):
    qt = np.asarray(qt, dtype=np.float32)
    kt = np.asarray(kt, dtype=np.float32)
    vt = np.asarray(vt, dtype=np.float32)
    wT = np.ascontiguousarray(np.asarray(weight, dtype=np.float32).T)  # [DK, M]
    maps = []
    for b in range(B):
        maps.append(
            {
                "qt": np.ascontiguousarray(qt[b]),
                "kt": np.ascontiguousarray(kt[b]),
                "vtT": np.ascontiguousarray(vt[b].T),  # [N2, d_v]
                "wT": wT,
            }
        )
    return maps


def kernel_with_result(qt, kt, vt, weight, **run_kwargs):
    """Like kernel() but also returns the BassKernelResults (for profiling)."""
    nc = _get_nc()
    res = run_bass_kernel_spmd(nc, _in_maps(qt, kt, vt, weight),
                               core_ids=list(range(B)), **run_kwargs)
    output = np.stack([np.asarray(res.results[b]["out"]) for b in range(B)])
    attn = np.stack([np.asarray(res.results[b]["attn"]) for b in range(B)])
    return (output.astype(np.float32, copy=False),
            attn.astype(np.float32, copy=False), res)


def kernel(qt, kt, vt, weight):
    """Full-input entry point: shards over B across 8 cores, returns
    (output [B, N, d_v], attn [B, N, N2]) matching the reference."""
    out, attn, _ = kernel_with_result(qt, kt, vt, weight)
    return out, attn


# revision 32
# speedup vs baseline: 1.1059x; 1.0759x over previous
"""Trainium2 Bass kernel for nn_MixtureOfSoftMaxACF (mixture-of-softmax attention).

Reference computation per batch element b (B=8, one per NeuronCore):
    pi      = softmax(weight @ mean(qt[b], axis=-1))                      # [m=2]
    A_j     = q_j^T k_j / sqrt(d_k)       (q_j, k_j = d=256-row slices)   # [N, N2]
    attn    = sum_j pi_j * softmax(A_j, axis=-1)                          # [N, N2]
    out     = attn @ vt[b]^T                                              # [N, d_v]

Sharding: data-parallel over B across the 8 cores. Inside each core:
  - QK^T runs in bf16 with native [d_k, N] layouts (lhsT=q, rhs=k).
  - exp on ScalarE with accum_out giving row sums for free (no max
    subtraction needed: |logits| <= ~6).
  - mixture weights pi computed on-device from the q-cast accumulation.
  - mixing on VectorE with per-partition scalars r_j = pi_j / S_j.
  - attn is transposed (DMA xbar or PE) to feed the attn @ v matmul.
"""

import sys

if "/opt/trn_rl_repo" not in sys.path:
    sys.path.insert(0, "/opt/trn_rl_repo")

import numpy as np
from contextlib import ExitStack

import concourse.bacc as bacc
import concourse.bass as bass
import concourse.tile as tile
from concourse import mybir
from concourse.bass_utils import run_bass_kernel_spmd
from concourse.masks import make_identity

B, DK, N = 8, 512, 2048
M = 2                       # mixture components
D = DK // M                 # 256 per-mixture head dim
TEMP = float(DK) ** 0.5     # sqrt(d_k), matching the reference
NCH = N // 128              # 16 n-chunks (and n2-chunks)
DCH = DK // 128             # 4 d_k partition chunks

f32 = mybir.dt.float32
bf16 = mybir.dt.bfloat16
f8 = mybir.dt.float8e4
F8_SCALE = 64.0  # lift attn probs out of fp8e4m3 subnormals; attn <= 1 so 64x never overflows
Exp = mybir.ActivationFunctionType.Exp
Copy = mybir.ActivationFunctionType.Copy
MULT = mybir.AluOpType.mult
ADD = mybir.AluOpType.add

TRANSPOSE_MODE = "pe"     # "xbar" (DMA xbar transpose) or "pe" (TensorE transpose)

_NC_CACHE = {}


def _body(ctx, tc, qt, kt, vtT, wT, out_d, attn_d, mode):
    nc = tc.nc

    const = ctx.enter_context(tc.tile_pool(name="const", bufs=1))
    resid = ctx.enter_context(tc.tile_pool(name="resid", bufs=1))
    stage = ctx.enter_context(tc.tile_pool(name="stage", bufs=2))
    stage_k = ctx.enter_context(tc.tile_pool(name="stage_k", bufs=4))
    stage_q = ctx.enter_context(tc.tile_pool(name="stage_q", bufs=4))
    epool = ctx.enter_context(tc.tile_pool(name="epool", bufs=6))
    apool = ctx.enter_context(tc.tile_pool(name="apool", bufs=3))
    spool = ctx.enter_context(tc.tile_pool(name="spool", bufs=6))
    opool = ctx.enter_context(tc.tile_pool(name="opool", bufs=3))
    # PSUM: psA 2 halves x [128,1024] (2 banks each) = 4 banks,
    # psT [128,512] = 1-2 banks, psU [128,512] x2 = 2 banks.
    psA_p = ctx.enter_context(tc.tile_pool(name="psA", bufs=2, space="PSUM"))
    psT_p = ctx.enter_context(
        tc.tile_pool(name="psT", bufs=(2 if mode == "pe" else 1), space="PSUM")
    )
    psU_p = ctx.enter_context(tc.tile_pool(name="psU", bufs=2, space="PSUM"))

    # ---- load + cast inputs ----
    # k loads first (every chunk needs all of k as the moving operand),
    # in column halves so chunk 0's first matmuls start ~5 us in. q loads
    # as 256-column blocks just-in-time for successive chunks, cast on
    # DVE with accum partials feeding the mixture-weight logits.
    q_bf = resid.tile([128, DCH, N], bf16)
    k_bf = resid.tile([128, DCH, N], bf16)
    qp = spool.tile([128, DCH, 8], f32, tag="qp")
    qr = qt[:, :].rearrange("(c p) n -> p c n", p=128)

    def _load_k_half(c, h):
        st = stage_k.tile([128, 1024], f32, tag="khalf", name=f"stk_{c}_{h}")
        nc.sync.dma_start(st, kt[bass.ts(c, 128), bass.ts(h, 1024)])
        nc.vector.tensor_copy(k_bf[:, c, bass.ts(h, 1024)], st)

    def _load_qcol(b):
        st = stage_q.tile([128, DCH, 256], f32, tag="qcol", name=f"stq_{b}")
        nc.sync.dma_start(st, qr[:, :, bass.ts(b, 256)])
        for c in range(DCH):
            nc.vector.tensor_scalar(
                q_bf[:, c, bass.ts(b, 256)], st[:, c, :], 1.0, 0.0, MULT, ADD,
                accum_out=qp[:, c, b : b + 1],
            )

    def _load_k_quarter(c, qtr):
        st = stage_k.tile([128, 512], f32, tag="kq", name=f"stkq_{c}_{qtr}")
        nc.sync.dma_start(st, kt[bass.ts(c, 128), bass.ts(qtr, 512)])
        nc.vector.tensor_copy(k_bf[:, c, bass.ts(qtr, 512)], st)

    vT_bf = resid.tile([128, NCH, DK], bf16)
    vr = vtT[:, :].rearrange("(g c p) d -> g p c d", g=4, p=128)

    def _load_v(g):
        st = stage.tile([128, 4, DK], f32, tag="stage", name=f"stv_{g}")
        nc.sync.dma_start(st, vr[g])
        nc.vector.tensor_copy(vT_bf[:, g * 4 : (g + 1) * 4, :], st)

    for c in (0, 1):
        _load_k_half(c, 0)
    _load_qcol(0)
    for c in (0, 1):
        _load_k_half(c, 1)
    for c in (2, 3):
        _load_k_half(c, 0)
    for c in (2, 3):
        _load_k_half(c, 1)
    for b in range(1, 8):
        _load_qcol(b)

    qs = spool.tile([128, DCH], f32, tag="qs")
    nc.vector.reduce_sum(qs, qp, axis=mybir.AxisListType.X)

    wT_sb = const.tile([128, DCH, M], f32)
    nc.sync.dma_start(wT_sb, wT[:, :].rearrange("(c p) m -> p c m", p=128))

    for g in range(4):
        _load_v(g)

    # ---- mixture weights pi ----
    # logits[m] = sum_d qsum[d] * wT[d, m]  (qsum = N * bar_q)
    lg_ps = psT_p.tile([128, 512], f32, tag="psT")
    for c in range(DCH):
        nc.tensor.matmul(
            lg_ps[:1, :M],
            lhsT=qs[:, c : c + 1],
            rhs=wT_sb[:, c, :],
            start=(c == 0),
            stop=(c == DCH - 1),
        )
    e_pi = spool.tile([1, M], f32, tag="epi")
    s_pi = spool.tile([1, 1], f32, tag="spi")
    # pi = softmax(logits / N); |logits/N| is tiny so no max subtraction
    nc.scalar.activation(e_pi, lg_ps[:1, :M], Exp, scale=1.0 / float(N), accum_out=s_pi)
    rs_pi = spool.tile([1, 1], f32, tag="rspi")
    nc.vector.reciprocal(rs_pi, s_pi)
    pi_row = spool.tile([1, M], f32, tag="pirow")
    nc.vector.tensor_scalar_mul(pi_row, e_pi, rs_pi)
    # broadcast pi to all 128 partitions via ones-matmul
    ones_r = const.tile([1, 128], f32)
    nc.vector.memset(ones_r, 1.0)
    pi_ps = psT_p.tile([128, 512], f32, tag="psT")
    nc.tensor.matmul(pi_ps[:, :M], lhsT=ones_r, rhs=pi_row, start=True, stop=True)
    pi_bc = const.tile([128, M], f32)
    nc.vector.tensor_copy(pi_bc, pi_ps[:, :M])

    if mode == "pe":
        ident = const.tile([128, 128], bf16)
        make_identity(nc, ident)

    # ---- main loop over 128-row n-chunks ----
    # Emission interleaves chunk ni's QK matmuls with chunk ni-1's
    # transpose+AV matmuls so the PE stream always has filler while the
    # ScalarE exps drain the QK PSUM tiles, and each transpose's
    # LDWEIGHTS hides behind an AV matmul's 512-col stream.
    prev = None

    def _emit_qk(ni, j, S2):
        e_t = epool.tile([128, N], bf16, tag="E", name=f"e_{ni}_{j}")
        for h in range(2):
            pa = psA_p.tile([128, 1024], f32, tag="psA", name=f"pa_{ni}_{j}_{h}")
            for d in range(2):
                dch = j * 2 + d
                lhsT = q_bf[:, dch, bass.ts(ni, 128)]
                for s in range(2):
                    nc.tensor.matmul(
                        pa[:, bass.ts(s, 512)],
                        lhsT=lhsT,
                        rhs=k_bf[:, dch, h * 1024 + s * 512 : h * 1024 + (s + 1) * 512],
                        start=(d == 0),
                        stop=(d == 1),
                    )
            nc.scalar.activation(
                e_t[:, bass.ts(h, 1024)], pa, Exp, scale=1.0 / TEMP,
                accum_out=S2[:, j : j + 1] if h == 1 else S2[:, 2 + j : 3 + j],
            )
        return e_t

    def _emit_mix(ni, E, S2):
        # S = partial(h0) + partial(h1); r_j = pi_j / S_j
        S = spool.tile([128, M], f32, tag="S", name=f"S_{ni}")
        nc.vector.tensor_add(S, S2[:, 0:M], S2[:, M : 2 * M])
        rS = spool.tile([128, M], f32, tag="rS", name=f"rS_{ni}")
        nc.vector.reciprocal(rS, S)
        r2 = spool.tile([128, M], f32, tag="r2", name=f"r2_{ni}")
        nc.vector.tensor_mul(r2, rS, pi_bc)
        t0_bf = apool.tile([128, N], bf16, tag="t0bf", name=f"t0_{ni}")
        nc.vector.tensor_scalar_mul(t0_bf, E[0], r2[:, 0:1])
        t_bf = apool.tile([128, N], bf16, tag="tbf", name=f"t1_{ni}")
        nc.vector.tensor_scalar_mul(t_bf, E[1], r2[:, 1:2])
        a_bf = apool.tile([128, N], bf16, tag="abf", name=f"abf_{ni}")
        nc.vector.tensor_add(a_bf, t0_bf, t_bf)
        a_f32 = apool.tile([128, N], f32, tag="af32", name=f"af32_{ni}")
        nc.vector.tensor_copy(a_f32, a_bf)
        nc.sync.dma_start(attn_d[bass.ts(ni, 128), :], a_f32)
        return a_bf

    class Chunk:
        pass

    def _emit_tav(ck, part):
        # part 0: transpose groups 0,1 + AV 0..3 ; part 1: the rest
        if part == 0:
            for g in (0, 1):
                pt = psT_p.tile([128, 512], bf16, tag="psT", name=f"pt_{ck.ni}_{g}")
                for kk in range(4):
                    nc.tensor.transpose(
                        pt[:, bass.ts(kk, 128)],
                        ck.a_bf[:, bass.ts(g * 4 + kk, 128)], ident,
                    )
                nc.vector.tensor_copy(ck.aT[:, bass.ts(g, 512)], pt)
            for kb in range(4):
                _av(ck, kb)
        else:
            for g in (2, 3):
                pt = psT_p.tile([128, 512], bf16, tag="psT", name=f"pt_{ck.ni}_{g}")
                for kk in range(4):
                    nc.tensor.transpose(
                        pt[:, bass.ts(kk, 128)],
                        ck.a_bf[:, bass.ts(g * 4 + kk, 128)], ident,
                    )
                nc.vector.tensor_copy(ck.aT[:, bass.ts(g, 512)], pt)
                for kb in range((g - 1) * 4, g * 4):
                    _av(ck, kb)
            for kb in range(12, 16):
                _av(ck, kb)
            o_t = opool.tile([128, DK], f32, tag="ot", name=f"ot_{ck.ni}")
            nc.scalar.copy(o_t, ck.pu)
            nc.sync.dma_start(out_d[bass.ts(ck.ni, 128), :], o_t)

    def _av(ck, kb):
        nc.tensor.matmul(
            ck.pu,
            lhsT=ck.aT[:, bass.ts(kb, 128)],
            rhs=vT_bf[:, kb, :],
            start=(kb == 0),
            stop=(kb == NCH - 1),
        )

    for ni in range(NCH):
        S2 = spool.tile([128, 2 * M], f32, tag="S2", name=f"S2_{ni}")
        E = [_emit_qk(ni, 0, S2)]
        if prev is not None:
            _emit_tav(prev, 0)
        E.append(_emit_qk(ni, 1, S2))
        if prev is not None:
            _emit_tav(prev, 1)
        a_bf = _emit_mix(ni, E, S2)
        ck = Chunk()
        ck.ni = ni
        ck.a_bf = a_bf
        ck.aT = apool.tile([128, N], bf16, tag="aT", name=f"aT_{ni}")
        ck.pu = psU_p.tile([128, DK], f32, tag="psU", name=f"pu_{ni}")
        prev = ck

    _emit_tav(prev, 0)
    _emit_tav(prev, 1)


def _build(mode):
    nc = bacc.Bacc()
    qt = nc.dram_tensor("qt", [DK, N], f32, kind="ExternalInput")
    kt = nc.dram_tensor("kt", [DK, N], f32, kind="ExternalInput")
    vtT = nc.dram_tensor("vtT", [N, DK], f32, kind="ExternalInput")
    wT = nc.dram_tensor("wT", [DK, M], f32, kind="ExternalInput")
    out_d = nc.dram_tensor("out", [N, DK], f32, kind="ExternalOutput")
    attn_d = nc.dram_tensor("attn", [N, N], f32, kind="ExternalOutput")
    with tile.TileContext(nc) as tc, ExitStack() as ctx:
        _body(ctx, tc, qt, kt, vtT, wT, out_d, attn_d, mode)
    nc.finalize()
    return nc


def _get_nc(mode=None):
    mode = mode or TRANSPOSE_MODE
    if mode not in _NC_CACHE:
        _NC_CACHE[mode] = _build(mode)
    return _NC_CACHE[mode]


def _in_maps(qt, kt, vt, weight):
    qt = np.asarray(qt, dtype=np.float32)
    kt = np.asarray(kt, dtype=np.float32)
    vt = np.asarray(vt, dtype=np.float32)
    wT = np.ascontiguousarray(np.asarray(weight, dtype=np.float32).T)  # [DK, M]
    maps = []
    for b in range(B):
        maps.append(
            {
                "qt": np.ascontiguousarray(qt[b]),
                "kt": np.ascontiguousarray(kt[b]),
                "vtT": np.ascontiguousarray(vt[b].T),  # [N2, d_v]
                "wT": wT,
            }
        )
    return maps


def kernel_with_result(qt, kt, vt, weight, **run_kwargs):
    """Like kernel() but also returns the BassKernelResults (for profiling)."""
    nc = _get_nc()
    res = run_bass_kernel_spmd(nc, _in_maps(qt, kt, vt, weight),
                               core_ids=list(range(B)), **run_kwargs)
    output = np.stack([np.asarray(res.results[b]["out"]) for b in range(B)])
    attn = np.stack([np.asarray(res.results[b]["attn"]) for b in range(B)])
    return (output.astype(np.float32, copy=False),
            attn.astype(np.float32, copy=False), res)


def kernel(qt, kt, vt, weight):
    """Full-input entry point: shards over B across 8 cores, returns
    (output [B, N, d_v], attn [B, N, N2]) matching the reference."""
    out, attn, _ = kernel_with_result(qt, kt, vt, weight)
    return out, attn
